# revision 21
# baseline (speedup 1.0000x reference)
"""Trainium2 Bass kernel: fused multi-head self-attention block (CrossAttention module).

Sharding: 8 cores, each handles one (batch, query-slice) pair:
  core c -> batch b = c // 4, query rows q0 = (c % 4) * 1024 .. +1024.
Each core computes K/V projections for its full batch (replicated across the 4
cores sharing a batch), Q projection for its query slice, all 8 heads of
attention for its queries, and the output projection for its rows.
Host folds the per-channel gammas into the (pre-transposed) weights, transposes
x once, and concatenates the per-core outputs.

On-chip dataflow (per core, all fp32 accumulation, bf16 operands):
  - xT / xTq staged resident in SBUF once (no inner-loop DMA)
  - kT[ko, n]  = WkT.T @ xT   (key channels on partitions)  -- JIT per head-pair
  - qT[qo, n]  = WqT.T @ xTq                                -- JIT per head-pair
  - v[k, vo]   = xT.T @ WvT, stored interleaved with a ones column per head
                 ("vone" [128, 8*65]) so the attention rowsum comes free
  - scoresT tile [key 128, q 512] = kT_h.T @ qT_h, two heads packed as PE
    row-tiles (K=64 each) into one 2-bank PSUM tile (concurrent execution)
  - E = exp(SCALE * scoresT) via ScalarE, PSUM -> SBUF ([128, 1024] per instr)
  - rT[dv(+rowsum), q] += vone_h.T @ E, accumulated over 32 key chunks in PSUM
  - normalize: rowsums copied to a [2, 512] tile, reciprocal_approx_fast (DVE),
    partition-broadcast via a K=2 ones-matmul on the PE, then one DVE multiply
    per head -- no slow RECIPROCAL, no GpSimd on the critical path
  - outT[do, q] = WoT.T @ rTn + bo, accumulated across head pairs in SBUF
  - the whole normalize+output-projection tail is emitted as deferred thunks
    interleaved into the NEXT (hp, qt) iteration's key loop, so the PE queue
    never stalls at an iteration boundary (keeps HAM at full clock)
"""

import os
import sys

import numpy as np

for _p in ("/opt/trn_rl_repo", "/root/.axon_site/_ro/trn_rl_repo"):
    if os.path.isdir(_p) and _p not in sys.path:
        sys.path.append(_p)

B, N, D = 2, 4096, 512
H, DH = 8, 64
SCALE = DH ** -0.5
NCORES = 8
QPC = (B * N) // NCORES  # 1024 query rows per core
P = 128
CD = D // P              # 4 contraction chunks of 128
KC = N // P              # 32 key chunks of 128
NT = N // 512            # 8 key-column tiles of 512
QT = QPC // 512          # 2 query tiles of 512
HP = H // 2              # 4 head pairs

_PROGRAM = None
LAST_RESULT = None

# DVE-exp offload (Schraudolph magic-add + shift-bitcast 2^i, cubic 2^f):
# exp(SCALE*s) = 2^(A2*s); u = t+MAGIC2 rounds t to int i in u's mantissa,
# (bits(u)<<23) = bits(2^i) (127 bias folded into MAGIC2), f = t-i in
# [-0.5,0.5], 2^f ~ C0+C1*f+C2*f^2+C3*f^3 (max rel err 1.4e-4, under the
# bf16 output quantization). Runs on the otherwise-idle VectorE to relieve
# the saturated ScalarE exp stream.
import math
A2 = SCALE * math.log2(math.e)
MAGIC2 = float(1.5 * 2 ** 23 + 127)
EC0, EC1, EC2, EC3 = 0.99995134, 0.69325305, 0.24225698, 0.05502927


def _build_program():
    import concourse.tile as tile
    from concourse import bacc, mybir

    f32 = mybir.dt.float32
    bf16 = mybir.dt.bfloat16
    i32 = mybir.dt.int32
    AF = mybir.ActivationFunctionType
    OP = mybir.AluOpType

    nc = bacc.Bacc("TRN2", target_bir_lowering=False, debug=False)

    xT_a = nc.dram_tensor("xT", [D, N], bf16, kind="ExternalInput").ap()
    xTq_a = nc.dram_tensor("xTq", [D, QPC], bf16, kind="ExternalInput").ap()
    wq_a = nc.dram_tensor("wqT", [D, D], bf16, kind="ExternalInput").ap()
    wk_a = nc.dram_tensor("wkT", [D, D], bf16, kind="ExternalInput").ap()
    wv_a = nc.dram_tensor("wvT", [D, D], bf16, kind="ExternalInput").ap()
    wo_a = nc.dram_tensor("woT", [D, D], bf16, kind="ExternalInput").ap()
    bo_a = nc.dram_tensor("bo", [D], f32, kind="ExternalInput").ap()
    outT_a = nc.dram_tensor("outT", [D, QPC], f32, kind="ExternalOutput").ap()

    with tile.TileContext(nc) as tc:
        with (
            tc.tile_pool(name="w", bufs=1) as wpool,
            tc.tile_pool(name="xr", bufs=1) as xr,
            tc.tile_pool(name="kT", bufs=2) as kTp,
            tc.tile_pool(name="qT", bufs=2) as qTp,
            tc.tile_pool(name="vone", bufs=1) as vpool,
            tc.tile_pool(name="et", bufs=6) as etp,
            tc.tile_pool(name="dx", bufs=1) as dxp,
            tc.tile_pool(name="rTn", bufs=1) as rTnp,
            tc.tile_pool(name="ot", bufs=2) as otp,
            tc.tile_pool(name="nrm", bufs=2) as nrm,
            tc.tile_pool(name="acc", bufs=2, space="PSUM") as psa,
            tc.tile_pool(name="sc", bufs=2, space="PSUM") as pss,
        ):
            # ---- resident inputs: x (transposed), q-slice of x, weights.
            # One coalesced DMA per 512-column chunk / weight, emitted in
            # dependency order (wk+wq+xTq0+wv ahead of the bulk of xT) and
            # spread over both hardware DGE queues (SP + ACT) so the first
            # scores matmul can issue a few microseconds in.
            xT_sb = xr.tile([P, CD * N], bf16, tag="xT")
            xTq_sb = xr.tile([P, CD * QPC], bf16, tag="xTq")
            xT_src = xT_a.rearrange("(c p) n -> p c n", p=P)
            xT_dst = xT_sb[:].rearrange("p (c n) -> p c n", c=CD)
            xTq_src = xTq_a.rearrange("(c p) n -> p c n", p=P)
            xTq_dst = xTq_sb[:].rearrange("p (c n) -> p c n", c=CD)

            def wtile(tag):
                return wpool.tile([P, CD * 512], bf16, tag=tag, name=tag)

            wk, wq, wv, wo = wtile("wk"), wtile("wq"), wtile("wvo"), wtile("wo")
            bo_t = wpool.tile([P, CD], f32, tag="bo")

            def w_dma(eng, w, dram_ap, lo, hi):
                eng.dma_start(
                    w[:].rearrange("p (c n) -> p c n", c=CD)[:, :, lo:hi],
                    dram_ap.rearrange("(c p) n -> p c n", p=P)[:, :, lo:hi],
                )

            # Per-queue DMA is ~90-110 GB/s, so three DMA-issuing queues
            # (SP + ACT hardware DGE, GpSimd software DGE) are loaded in
            # strict need-order: head-pair-0 slices of wq/wk and the first x
            # chunks (split in cd halves so the first proj matmuls can chase
            # the DMAs) ahead of everything else.
            nc.sync.dma_start(bo_t[:], bo_a.rearrange("(c p) -> p c", p=P))
            w_dma(nc.sync, wq, wq_a, 0, P)          # q proj, hp0 slice
            nc.scalar.dma_start(xTq_dst[:, 0:2, 0:512], xTq_src[:, 0:2, 0:512])
            w_dma(nc.sync, wk, wk_a, 0, P)          # k proj, hp0 slice
            nc.scalar.dma_start(xTq_dst[:, 2:4, 0:512], xTq_src[:, 2:4, 0:512])
            w_dma(nc.gpsimd, wv, wv_a, 0, D)        # full v weights (vproj kc0)
            nc.sync.dma_start(xT_dst[:, 0:2, 0:512], xT_src[:, 0:2, 0:512])
            nc.sync.dma_start(xT_dst[:, 2:4, 0:512], xT_src[:, 2:4, 0:512])
            nc.scalar.dma_start(xT_dst[:, :, 512:1024], xT_src[:, :, 512:1024])
            nc.gpsimd.dma_start(xTq_dst[:, :, 512:1024], xTq_src[:, :, 512:1024])
            nc.sync.dma_start(xT_dst[:, :, 1024:1536], xT_src[:, :, 1024:1536])
            nc.scalar.dma_start(xT_dst[:, :, 1536:2048], xT_src[:, :, 1536:2048])
            nc.gpsimd.dma_start(xT_dst[:, :, 2048:2560], xT_src[:, :, 2048:2560])
            nc.sync.dma_start(xT_dst[:, :, 2560:3072], xT_src[:, :, 2560:3072])
            nc.scalar.dma_start(xT_dst[:, :, 3072:3584], xT_src[:, :, 3072:3584])
            w_dma(nc.gpsimd, wq, wq_a, P, D)        # remaining head pairs
            nc.sync.dma_start(xT_dst[:, :, 3584:4096], xT_src[:, :, 3584:4096])
            w_dma(nc.scalar, wk, wk_a, P, D)
            w_dma(nc.gpsimd, wo, wo_a, 0, D)

            def dve_exp(sp, et):
                """8-pass VectorE exp(SCALE*x): see module docstring constants."""
                u = dxp.tile([P, 1024], f32, tag="a", name="dveu")
                nc.vector.tensor_scalar(u[:], sp[:], A2, MAGIC2, OP.mult, OP.add)
                i_ = dxp.tile([P, 1024], f32, tag="b", name="dvei")
                nc.vector.tensor_scalar(i_[:], u[:], MAGIC2, None, OP.subtract)
                fT = dxp.tile([P, 1024], f32, tag="c", name="dvef")
                nc.vector.scalar_tensor_tensor(
                    fT[:], sp[:], A2, i_[:], OP.mult, OP.subtract)
                pb = dxp.tile([P, 1024], i32, tag="d", name="dvepb")
                nc.vector.tensor_scalar(
                    pb[:], u[:].bitcast(i32), 23, None, OP.logical_shift_left)
                h1 = dxp.tile([P, 1024], f32, tag="a", name="dveh1")
                nc.vector.tensor_scalar(h1[:], fT[:], EC3, EC2, OP.mult, OP.add)
                h2 = dxp.tile([P, 1024], f32, tag="b", name="dveh2")
                nc.vector.tensor_tensor(h2[:], h1[:], fT[:], op=OP.mult)
                h4 = dxp.tile([P, 1024], f32, tag="a", name="dveh4")
                nc.vector.scalar_tensor_tensor(
                    h4[:], h2[:], EC1, fT[:], OP.add, OP.mult)
                nc.vector.scalar_tensor_tensor(
                    et[:], h4[:], EC0, pb[:].bitcast(f32), OP.add, OP.mult)

            vones = [None] * KC
            oaccs = {}
            rTns = [
                rTnp.tile([P, QPC], bf16, tag=f"rTn{c}", name=f"rTn{c}")
                for c in range(CD)
            ]

            def proj_group(w_t, hp, src_sb, nt, dst):
                """One 512-wide output block of a W.T @ x projection:
                4 accumulating matmuls off the resident x, 1 evacuation."""
                src_stride = N if src_sb is xT_sb else QPC
                ps = psa.tile([P, 512], f32, tag="acc")
                for cd in range(CD):
                    nc.tensor.matmul(
                        ps[:],
                        w_t[:, cd * 512 + hp * P: cd * 512 + (hp + 1) * P],
                        src_sb[:, cd * src_stride + nt * 512:
                               cd * src_stride + (nt + 1) * 512],
                        start=(cd == 0),
                        stop=(cd == CD - 1),
                    )
                nc.vector.tensor_copy(dst, ps[:])

            def vproj_group(kc):
                """v projection for one 128-key chunk, written into the
                per-head [64 v | 1 ones] interleaved layout."""
                ps = psa.tile([P, 512], f32, tag="acc")
                for cd in range(CD):
                    nc.tensor.matmul(
                        ps[:],
                        xT_sb[:, cd * N + kc * P: cd * N + (kc + 1) * P],
                        wv[:, cd * 512:(cd + 1) * 512],
                        start=(cd == 0),
                        stop=(cd == CD - 1),
                    )
                vt = vpool.tile([P, H * 65], bf16, tag=f"vone{kc}")
                v3 = vt[:].rearrange("p (h c) -> p h c", c=65)
                nc.vector.tensor_copy(
                    v3[:, :, 0:64], ps[:].rearrange("p (h c) -> p h c", c=64)
                )
                nc.vector.memset(v3[:, :, 64:65], 1.0)
                vones[kc] = vt

            def make_proj_thunks(hp):
                qt_t = qTp.tile([P, QPC], bf16, tag="qT")
                kt_t = kTp.tile([P, N], bf16, tag="kT")
                thunks = []
                for nt in range(QT):
                    thunks.append(
                        lambda nt=nt, qt_t=qt_t, hp=hp: proj_group(
                            wq, hp, xTq_sb, nt, qt_t[:, nt * 512:(nt + 1) * 512]
                        )
                    )
                for nt in range(NT):
                    thunks.append(
                        lambda nt=nt, kt_t=kt_t, hp=hp: proj_group(
                            wk, hp, xT_sb, nt, kt_t[:, nt * 512:(nt + 1) * 512]
                        )
                    )
                return qt_t, kt_t, thunks

            def make_tail(hp, qt, rsbA, rsbB, srec):
                """Deferred normalize + output-projection thunks for (hp, qt).
                Emitted interleaved into the next iteration's key loop so the
                PE queue keeps streaming scores/AV while the DVE chain runs."""
                thunks = []

                def bcast(hp=hp, qt=qt, srec=srec):
                    # broadcast 1/rowsum across partitions on the (otherwise
                    # idle) GpSimd engine; latency hidden by the deferred-tail
                    # interleaving.
                    bcA = nrm.tile([64, 512], f32, tag="bcA", bufs=1, name="bcA")
                    nc.gpsimd.partition_broadcast(bcA[:], srec[0:1, 0:512])
                    bcB = nrm.tile([64, 512], f32, tag="bcB", bufs=1, name="bcB")
                    nc.gpsimd.partition_broadcast(bcB[:], srec[0:1, 512:1024])
                    for rsb, bc, poff in ((rsbA, bcA, 0), (rsbB, bcB, 64)):
                        nc.vector.tensor_tensor(
                            rTns[hp][poff:poff + 64, qt * 512:(qt + 1) * 512],
                            rsb[0:64, :],
                            bc[:],
                            op=OP.mult,
                        )

                thunks.append(bcast)

                def oproj(doc, hp=hp, qt=qt):
                    ps = psa.tile([P, 512], f32, tag="acc", name="ops")
                    nc.tensor.matmul(
                        ps[:],
                        wo[:, hp * 512 + doc * P: hp * 512 + (doc + 1) * P],
                        rTns[hp][:, qt * 512:(qt + 1) * 512],
                        start=True, stop=True,
                    )
                    if hp == 0:
                        oa = otp.tile([P, 512], f32, tag=f"oacc{qt}{doc}",
                                      bufs=1, name=f"oacc{qt}{doc}")
                        oaccs[(qt, doc)] = oa
                        nc.vector.tensor_copy(oa[:], ps[:])
                    else:
                        oa = oaccs[(qt, doc)]
                        nc.vector.tensor_tensor(oa[:], oa[:], ps[:], op=OP.add)
                    if hp == HP - 1:
                        ot = otp.tile([P, 512], f32, tag="ot")
                        nc.vector.tensor_tensor(
                            ot[:],
                            oa[:],
                            bo_t[:, doc:doc + 1].to_broadcast((P, 512)),
                            op=OP.add,
                        )
                        nc.sync.dma_start(
                            outT_a[doc * P:(doc + 1) * P,
                                   qt * 512:(qt + 1) * 512],
                            ot[:],
                        )

                for doc in range(CD):
                    thunks.append(lambda doc=doc: oproj(doc))
                return thunks

            qts, kts = {}, {}
            qts[0], kts[0], th0 = make_proj_thunks(0)
            # Emit only the blocks needed to start attention: qT block 0 and
            # kT block 0; the rest of hp0's projections interleave into the
            # first kc loop (kT block g must land before kc reaches 4g).
            th0[0]()
            th0[QT]()
            hp0_qt1_proj = th0[1:QT]
            hp0_kt = th0[QT + 1:]
            pending = []
            tail = []
            TAIL_KCS = (3, 5, 7, 9, 11)

            for hp in range(HP):
                qt_t, kt_t = qts[hp], kts[hp]
                for t in pending:  # leftover projections for this head pair
                    t()
                pending = []
                h0, h1 = 2 * hp, 2 * hp + 1
                for qt in range(QT):
                    rA = psa.tile([P, 512], f32, tag="rA", bufs=1, name="rA")
                    rB = psa.tile([P, 512], f32, tag="rB", bufs=1, name="rB")
                    if qt == 1 and hp + 1 < HP:
                        qts[hp + 1], kts[hp + 1], pending = make_proj_thunks(hp + 1)
                    qA = qt_t[0:64, qt * 512:(qt + 1) * 512]
                    qB = qt_t[64:128, qt * 512:(qt + 1) * 512]
                    # Software-pipelined by one chunk: emit scores(kc) and its
                    # exp, then the AV matmuls for kc-1 — so the PE always has
                    # independent score work queued while ScalarE runs exp.
                    # A few exp tiles per pass run on VectorE instead
                    # (dve_exp); their AV is deferred 4 chunks for pipeline
                    # depth. None in (hp0,qt0)/(hp0,qt1) where VectorE is
                    # busy with v/kt/qt-projection evacuations.
                    if hp == 0:
                        dve_kcs = () if qt == 0 else (16, 22)
                    else:
                        dve_kcs = (14, 18, 22, 26)
                    ets = {}
                    av_due = []  # (kc, due-iteration), emission-ordered

                    def av_pair(kc, rA=rA, rB=rB, ets=ets, h0=h0, h1=h1):
                        vt = vones[kc]
                        et = ets.pop(kc)
                        nc.tensor.matmul(
                            rA[0:65, :],
                            vt[:, h0 * 65:(h0 + 1) * 65],
                            et[:, 0:512],
                            start=(kc == 0), stop=(kc == KC - 1),
                        )
                        nc.tensor.matmul(
                            rB[0:65, :],
                            vt[:, h1 * 65:(h1 + 1) * 65],
                            et[:, 512:1024],
                            start=(kc == 0), stop=(kc == KC - 1),
                        )

                    for kc in range(KC):
                        sp = pss.tile([P, 1024], f32, tag="sc")
                        nc.tensor.matmul(
                            sp[:, 0:512],
                            kt_t[0:64, kc * P:(kc + 1) * P],
                            qA,
                            start=True, stop=True,
                            tile_position=(0, 0),
                        )
                        nc.tensor.matmul(
                            sp[:, 512:1024],
                            kt_t[64:128, kc * P:(kc + 1) * P],
                            qB,
                            start=True, stop=True,
                            tile_position=(64, 0),
                        )
                        et = etp.tile([P, 1024], bf16, tag="et")
                        if kc in dve_kcs:
                            dve_exp(sp, et)
                            av_due.append((kc, kc + 4))
                        else:
                            nc.scalar.activation(et[:], sp[:], AF.Exp,
                                                 scale=float(SCALE))
                            av_due.append((kc, kc + 1))
                        ets[kc] = et
                        if hp == 0 and qt == 0:
                            # JIT v projection: vone[kc] lands just ahead of
                            # av_pair(kc); emitting it after scores/exp keeps
                            # the first exp off the wv-DMA critical path.
                            vproj_group(kc)
                            if hp0_kt and kc % 4 == 2:
                                hp0_kt.pop(0)()
                            if kc == 24:
                                for t in hp0_qt1_proj:
                                    t()
                                hp0_qt1_proj = []
                        for ent in [e for e in av_due if e[1] <= kc]:
                            av_due.remove(ent)
                            av_pair(ent[0])
                        if tail and kc in TAIL_KCS:
                            tail.pop(0)()
                        if pending and kc % 3 == 2:
                            pending.pop(0)()
                    for ent in av_due:  # drain (ends with kc=KC-1: stop flag)
                        av_pair(ent[0])
                    for t in tail:  # safety drain (normally empty by now)
                        t()
                    tail = []
                    # Evacuate both accumulators to SBUF immediately (frees
                    # the PSUM slots); rowsums go to a [2, 512] tile for one
                    # fast approximate reciprocal over both heads.
                    rsbA = nrm.tile([65, 512], f32, tag="rsb", bufs=4, name="rsbA")
                    nc.vector.tensor_copy(rsbA[:], rA[0:65, :])
                    rsbB = nrm.tile([65, 512], f32, tag="rsb", bufs=4, name="rsbB")
                    nc.vector.tensor_copy(rsbB[:], rB[0:65, :])
                    scr = nrm.tile([1, 1024], f32, tag="scr", bufs=1, name="scr")
                    nc.vector.tensor_copy(scr[0:1, 0:512], rsbA[64:65, :])
                    nc.vector.tensor_copy(scr[0:1, 512:1024], rsbB[64:65, :])
                    srec = nrm.tile([1, 1024], f32, tag="srec", bufs=2, name="srec")
                    nc.vector.reciprocal_approx_fast(srec[:], scr[:])
                    tail = make_tail(hp, qt, rsbA, rsbB, srec)

            for t in tail:  # final iteration's tail
                t()

    nc.compile()
    return nc


def _get_program():
    global _PROGRAM
    if _PROGRAM is None:
        _PROGRAM = _build_program()
    return _PROGRAM


def kernel(x, Wq, Wk, Wv, Wo, bo, gamma_q, gamma_k, gamma_v, gamma_out):
    from concourse import bass_utils

    import ml_dtypes

    bf16 = ml_dtypes.bfloat16
    x = np.asarray(x, dtype=np.float32)
    f32 = np.float32
    WqT = np.ascontiguousarray((np.asarray(Wq, f32).T * np.asarray(gamma_q, f32)[None, :]).astype(bf16))
    WkT = np.ascontiguousarray((np.asarray(Wk, f32).T * np.asarray(gamma_k, f32)[None, :]).astype(bf16))
    WvT = np.ascontiguousarray((np.asarray(Wv, f32).T * np.asarray(gamma_v, f32)[None, :]).astype(bf16))
    WoT = np.ascontiguousarray((np.asarray(Wo, f32).T * np.asarray(gamma_out, f32)[None, :]).astype(bf16))
    bo_s = np.ascontiguousarray(np.asarray(gamma_out, f32) * np.asarray(bo, f32))

    xT = np.ascontiguousarray(x.transpose(0, 2, 1).astype(bf16))  # [B, D, N]

    in_maps = []
    for c in range(NCORES):
        b, q0 = c // 4, (c % 4) * QPC
        in_maps.append({
            "xT": xT[b],
            "xTq": np.ascontiguousarray(xT[b][:, q0:q0 + QPC]),
            "wqT": WqT, "wkT": WkT, "wvT": WvT, "woT": WoT,
            "bo": bo_s,
        })

    nc = _get_program()
    res = bass_utils.run_bass_kernel_spmd(nc, in_maps, core_ids=list(range(NCORES)))
    global LAST_RESULT
    LAST_RESULT = res

    out = np.empty((B, N, D), np.float32)
    for c in range(NCORES):
        b, q0 = c // 4, (c % 4) * QPC
        out[b, q0:q0 + QPC, :] = res.results[c]["outT"].T
    return out


# revision 22
# speedup vs baseline: 1.3038x; 1.3038x over previous
"""Trainium2 Bass kernel: fused multi-head self-attention block (CrossAttention module).

Sharding: 8 cores, each handles one (batch, query-slice) pair:
  core c -> batch b = c // 4, query rows q0 = (c % 4) * 1024 .. +1024.
Each core computes K/V projections for its full batch (replicated across the 4
cores sharing a batch), Q projection for its query slice, all 8 heads of
attention for its queries, and the output projection for its rows.
Host folds the per-channel gammas into the (pre-transposed) weights, transposes
x once, and concatenates the per-core outputs.

On-chip dataflow (per core, all fp32 accumulation, bf16 operands):
  - xT / xTq staged resident in SBUF once (no inner-loop DMA)
  - kT[ko, n]  = WkT.T @ xT   (key channels on partitions)  -- JIT per head-pair
  - qT[qo, n]  = WqT.T @ xTq                                -- JIT per head-pair
  - v[k, vo]   = xT.T @ WvT, stored interleaved with a ones column per head
                 ("vone" [128, 8*65]) so the attention rowsum comes free
  - scoresT tile [key 128, q 512] = kT_h.T @ qT_h, two heads packed as PE
    row-tiles (K=64 each) into one 2-bank PSUM tile (concurrent execution)
  - E = exp(SCALE * scoresT) via ScalarE, PSUM -> SBUF ([128, 1024] per instr)
  - rT[dv(+rowsum), q] += vone_h.T @ E, accumulated over 32 key chunks in PSUM
  - normalize: rowsums copied to a [2, 512] tile, reciprocal_approx_fast (DVE),
    partition-broadcast via a K=2 ones-matmul on the PE, then one DVE multiply
    per head -- no slow RECIPROCAL, no GpSimd on the critical path
  - outT[do, q] = WoT.T @ rTn + bo, accumulated across head pairs in SBUF
  - the whole normalize+output-projection tail is emitted as deferred thunks
    interleaved into the NEXT (hp, qt) iteration's key loop, so the PE queue
    never stalls at an iteration boundary (keeps HAM at full clock)
"""

import os
import sys

import numpy as np

for _p in ("/opt/trn_rl_repo", "/root/.axon_site/_ro/trn_rl_repo"):
    if os.path.isdir(_p) and _p not in sys.path:
        sys.path.append(_p)

B, N, D = 2, 4096, 512
H, DH = 8, 64
SCALE = DH ** -0.5
NCORES = 8
QPC = (B * N) // NCORES  # 1024 query rows per core
P = 128
CD = D // P              # 4 contraction chunks of 128
KC = N // P              # 32 key chunks of 128
NT = N // 512            # 8 key-column tiles of 512
QT = QPC // 512          # 2 query tiles of 512
HP = H // 2              # 4 head pairs

_PROGRAM = None
LAST_RESULT = None

# DVE-exp offload (Schraudolph magic-add + shift-bitcast 2^i, cubic 2^f):
# exp(SCALE*s) = 2^(A2*s); u = t+MAGIC2 rounds t to int i in u's mantissa,
# (bits(u)<<23) = bits(2^i) (127 bias folded into MAGIC2), f = t-i in
# [-0.5,0.5], 2^f ~ C0+C1*f+C2*f^2+C3*f^3 (max rel err 1.4e-4, under the
# bf16 output quantization). Runs on the otherwise-idle VectorE to relieve
# the saturated ScalarE exp stream.
import math
A2 = SCALE * math.log2(math.e)
MAGIC2 = float(1.5 * 2 ** 23 + 127)
EC0, EC1, EC2, EC3 = 0.99995134, 0.69325305, 0.24225698, 0.05502927


def _build_program():
    import concourse.tile as tile
    from concourse import bacc, mybir

    f32 = mybir.dt.float32
    bf16 = mybir.dt.bfloat16
    i32 = mybir.dt.int32
    AF = mybir.ActivationFunctionType
    OP = mybir.AluOpType

    nc = bacc.Bacc("TRN2", target_bir_lowering=False, debug=False)

    xT_a = nc.dram_tensor("xT", [D, N], bf16, kind="ExternalInput").ap()
    xTq_a = nc.dram_tensor("xTq", [D, QPC], bf16, kind="ExternalInput").ap()
    wq_a = nc.dram_tensor("wqT", [D, D], bf16, kind="ExternalInput").ap()
    wk_a = nc.dram_tensor("wkT", [D, D], bf16, kind="ExternalInput").ap()
    wv_a = nc.dram_tensor("wvT", [D, D], bf16, kind="ExternalInput").ap()
    wo_a = nc.dram_tensor("woT", [D, D], bf16, kind="ExternalInput").ap()
    bo_a = nc.dram_tensor("bo", [D], f32, kind="ExternalInput").ap()
    outT_a = nc.dram_tensor("outT", [D, QPC], f32, kind="ExternalOutput").ap()

    with tile.TileContext(nc) as tc:
        with (
            tc.tile_pool(name="w", bufs=1) as wpool,
            tc.tile_pool(name="xr", bufs=1) as xr,
            tc.tile_pool(name="kT", bufs=2) as kTp,
            tc.tile_pool(name="qT", bufs=2) as qTp,
            tc.tile_pool(name="vone", bufs=1) as vpool,
            tc.tile_pool(name="et", bufs=6) as etp,
            tc.tile_pool(name="dx", bufs=1) as dxp,
            tc.tile_pool(name="rTn", bufs=1) as rTnp,
            tc.tile_pool(name="ot", bufs=2) as otp,
            tc.tile_pool(name="nrm", bufs=2) as nrm,
            tc.tile_pool(name="acc", bufs=2, space="PSUM") as psa,
            tc.tile_pool(name="sc", bufs=2, space="PSUM") as pss,
        ):
            # ---- resident inputs: x (transposed), q-slice of x, weights.
            # One coalesced DMA per 512-column chunk / weight, emitted in
            # dependency order (wk+wq+xTq0+wv ahead of the bulk of xT) and
            # spread over both hardware DGE queues (SP + ACT) so the first
            # scores matmul can issue a few microseconds in.
            xT_sb = xr.tile([P, CD * N], bf16, tag="xT")
            xTq_sb = xr.tile([P, CD * QPC], bf16, tag="xTq")
            xT_src = xT_a.rearrange("(c p) n -> p c n", p=P)
            xT_dst = xT_sb[:].rearrange("p (c n) -> p c n", c=CD)
            xTq_src = xTq_a.rearrange("(c p) n -> p c n", p=P)
            xTq_dst = xTq_sb[:].rearrange("p (c n) -> p c n", c=CD)

            def wtile(tag):
                return wpool.tile([P, CD * 512], bf16, tag=tag, name=tag)

            wk, wq, wv, wo = wtile("wk"), wtile("wq"), wtile("wvo"), wtile("wo")
            bo_t = wpool.tile([P, CD], f32, tag="bo")

            def w_dma(eng, w, dram_ap, lo, hi):
                eng.dma_start(
                    w[:].rearrange("p (c n) -> p c n", c=CD)[:, :, lo:hi],
                    dram_ap.rearrange("(c p) n -> p c n", p=P)[:, :, lo:hi],
                )

            # Per-queue DMA is ~90-110 GB/s, so three DMA-issuing queues
            # (SP + ACT hardware DGE, GpSimd software DGE) are loaded in
            # strict need-order: head-pair-0 slices of wq/wk and the first x
            # chunks (split in cd halves so the first proj matmuls can chase
            # the DMAs) ahead of everything else.
            w_dma(nc.sync, wq, wq_a, 0, P)          # q proj, hp0 slice
            nc.scalar.dma_start(xTq_dst[:, 0:2, 0:512], xTq_src[:, 0:2, 0:512])
            w_dma(nc.sync, wk, wk_a, 0, P)          # k proj, hp0 slice
            nc.scalar.dma_start(xTq_dst[:, 2:4, 0:512], xTq_src[:, 2:4, 0:512])
            nc.sync.dma_start(xT_dst[:, 0:2, 0:512], xT_src[:, 0:2, 0:512])
            w_dma(nc.scalar, wv, wv_a, 0, P)        # v weights in cd halves
            nc.sync.dma_start(xT_dst[:, 2:4, 0:512], xT_src[:, 2:4, 0:512])
            w_dma(nc.scalar, wv, wv_a, P, D)
            nc.sync.dma_start(bo_t[:], bo_a.rearrange("(c p) -> p c", p=P))
            nc.gpsimd.dma_start(xT_dst[:, :, 2048:2560], xT_src[:, :, 2048:2560])
            nc.sync.dma_start(xT_dst[:, :, 512:1024], xT_src[:, :, 512:1024])
            nc.scalar.dma_start(xT_dst[:, :, 1024:1536], xT_src[:, :, 1024:1536])
            nc.gpsimd.dma_start(xT_dst[:, :, 2560:3072], xT_src[:, :, 2560:3072])
            nc.sync.dma_start(xT_dst[:, :, 1536:2048], xT_src[:, :, 1536:2048])
            nc.scalar.dma_start(xTq_dst[:, :, 512:1024], xTq_src[:, :, 512:1024])
            nc.gpsimd.dma_start(xT_dst[:, :, 3072:3584], xT_src[:, :, 3072:3584])
            w_dma(nc.sync, wq, wq_a, P, D)          # remaining head pairs
            w_dma(nc.scalar, wk, wk_a, P, D)
            nc.gpsimd.dma_start(xT_dst[:, :, 3584:4096], xT_src[:, :, 3584:4096])
            w_dma(nc.gpsimd, wo, wo_a, 0, D)

            def dve_exp(sp, et):
                """8-pass VectorE exp(SCALE*x): see module docstring constants."""
                u = dxp.tile([P, 1024], f32, tag="a", name="dveu")
                nc.vector.tensor_scalar(u[:], sp[:], A2, MAGIC2, OP.mult, OP.add)
                i_ = dxp.tile([P, 1024], f32, tag="b", name="dvei")
                nc.vector.tensor_scalar(i_[:], u[:], MAGIC2, None, OP.subtract)
                fT = dxp.tile([P, 1024], f32, tag="c", name="dvef")
                nc.vector.scalar_tensor_tensor(
                    fT[:], sp[:], A2, i_[:], OP.mult, OP.subtract)
                pb = dxp.tile([P, 1024], i32, tag="d", name="dvepb")
                nc.vector.tensor_scalar(
                    pb[:], u[:].bitcast(i32), 23, None, OP.logical_shift_left)
                h1 = dxp.tile([P, 1024], f32, tag="a", name="dveh1")
                nc.vector.tensor_scalar(h1[:], fT[:], EC3, EC2, OP.mult, OP.add)
                h2 = dxp.tile([P, 1024], f32, tag="b", name="dveh2")
                nc.vector.tensor_tensor(h2[:], h1[:], fT[:], op=OP.mult)
                h4 = dxp.tile([P, 1024], f32, tag="a", name="dveh4")
                nc.vector.scalar_tensor_tensor(
                    h4[:], h2[:], EC1, fT[:], OP.add, OP.mult)
                nc.vector.scalar_tensor_tensor(
                    et[:], h4[:], EC0, pb[:].bitcast(f32), OP.add, OP.mult)

            vones = [None] * KC
            oaccs = {}
            rTns = [
                rTnp.tile([P, QPC], bf16, tag=f"rTn{c}", name=f"rTn{c}")
                for c in range(CD)
            ]

            def proj_group(w_t, hp, src_sb, nt, dst):
                """One 512-wide output block of a W.T @ x projection:
                4 accumulating matmuls off the resident x, 1 evacuation."""
                src_stride = N if src_sb is xT_sb else QPC
                ps = psa.tile([P, 512], f32, tag="acc")
                for cd in range(CD):
                    nc.tensor.matmul(
                        ps[:],
                        w_t[:, cd * 512 + hp * P: cd * 512 + (hp + 1) * P],
                        src_sb[:, cd * src_stride + nt * 512:
                               cd * src_stride + (nt + 1) * 512],
                        start=(cd == 0),
                        stop=(cd == CD - 1),
                    )
                nc.vector.tensor_copy(dst, ps[:])

            def vproj_group(kc):
                """v projection for one 128-key chunk, written into the
                per-head [64 v | 1 ones] interleaved layout."""
                ps = psa.tile([P, 512], f32, tag="acc")
                for cd in range(CD):
                    nc.tensor.matmul(
                        ps[:],
                        xT_sb[:, cd * N + kc * P: cd * N + (kc + 1) * P],
                        wv[:, cd * 512:(cd + 1) * 512],
                        start=(cd == 0),
                        stop=(cd == CD - 1),
                    )
                vt = vpool.tile([P, H * 65], bf16, tag=f"vone{kc}")
                v3 = vt[:].rearrange("p (h c) -> p h c", c=65)
                nc.vector.tensor_copy(
                    v3[:, :, 0:64], ps[:].rearrange("p (h c) -> p h c", c=64)
                )
                nc.vector.memset(v3[:, :, 64:65], 1.0)
                vones[kc] = vt

            def make_proj_thunks(hp):
                qt_t = qTp.tile([P, QPC], bf16, tag="qT")
                kt_t = kTp.tile([P, N], bf16, tag="kT")
                thunks = []
                for nt in range(QT):
                    thunks.append(
                        lambda nt=nt, qt_t=qt_t, hp=hp: proj_group(
                            wq, hp, xTq_sb, nt, qt_t[:, nt * 512:(nt + 1) * 512]
                        )
                    )
                for nt in range(NT):
                    thunks.append(
                        lambda nt=nt, kt_t=kt_t, hp=hp: proj_group(
                            wk, hp, xT_sb, nt, kt_t[:, nt * 512:(nt + 1) * 512]
                        )
                    )
                return qt_t, kt_t, thunks

            def make_tail(hp, qt, rsbA, rsbB, srec):
                """Deferred normalize + output-projection thunks for (hp, qt).
                Emitted interleaved into the next iteration's key loop so the
                PE queue keeps streaming scores/AV while the DVE chain runs."""
                thunks = []

                def bcast(hp=hp, qt=qt, srec=srec):
                    # broadcast 1/rowsum across partitions on the (otherwise
                    # idle) GpSimd engine; latency hidden by the deferred-tail
                    # interleaving.
                    bcA = nrm.tile([64, 512], f32, tag="bcA", bufs=1, name="bcA")
                    nc.gpsimd.partition_broadcast(bcA[:], srec[0:1, 0:512])
                    bcB = nrm.tile([64, 512], f32, tag="bcB", bufs=1, name="bcB")
                    nc.gpsimd.partition_broadcast(bcB[:], srec[0:1, 512:1024])
                    for rsb, bc, poff in ((rsbA, bcA, 0), (rsbB, bcB, 64)):
                        nc.vector.tensor_tensor(
                            rTns[hp][poff:poff + 64, qt * 512:(qt + 1) * 512],
                            rsb[0:64, :],
                            bc[:],
                            op=OP.mult,
                        )

                thunks.append(bcast)

                def oproj(doc, hp=hp, qt=qt):
                    ps = psa.tile([P, 512], f32, tag="acc", name="ops")
                    nc.tensor.matmul(
                        ps[:],
                        wo[:, hp * 512 + doc * P: hp * 512 + (doc + 1) * P],
                        rTns[hp][:, qt * 512:(qt + 1) * 512],
                        start=True, stop=True,
                    )
                    if hp == 0:
                        oa = otp.tile([P, 512], f32, tag=f"oacc{qt}{doc}",
                                      bufs=1, name=f"oacc{qt}{doc}")
                        oaccs[(qt, doc)] = oa
                        nc.vector.tensor_copy(oa[:], ps[:])
                    else:
                        oa = oaccs[(qt, doc)]
                        nc.vector.tensor_tensor(oa[:], oa[:], ps[:], op=OP.add)
                    if hp == HP - 1:
                        ot = otp.tile([P, 512], f32, tag="ot")
                        nc.vector.tensor_tensor(
                            ot[:],
                            oa[:],
                            bo_t[:, doc:doc + 1].to_broadcast((P, 512)),
                            op=OP.add,
                        )
                        nc.sync.dma_start(
                            outT_a[doc * P:(doc + 1) * P,
                                   qt * 512:(qt + 1) * 512],
                            ot[:],
                        )

                for doc in range(CD):
                    thunks.append(lambda doc=doc: oproj(doc))
                return thunks

            qts, kts = {}, {}
            qts[0], kts[0], th0 = make_proj_thunks(0)
            # Emit only the blocks needed to start attention: qT block 0 and
            # kT block 0; the rest of hp0's projections interleave into the
            # first kc loop (kT block g must land before kc reaches 4g).
            th0[0]()
            th0[QT]()
            hp0_qt1_proj = th0[1:QT]
            hp0_kt = th0[QT + 1:]
            pending = []
            tail = []
            TAIL_KCS = (3, 5, 7, 9, 11)

            for hp in range(HP):
                qt_t, kt_t = qts[hp], kts[hp]
                for t in pending:  # leftover projections for this head pair
                    t()
                pending = []
                h0, h1 = 2 * hp, 2 * hp + 1
                for qt in range(QT):
                    rA = psa.tile([P, 512], f32, tag="rA", bufs=1, name="rA")
                    rB = psa.tile([P, 512], f32, tag="rB", bufs=1, name="rB")
                    if qt == 1 and hp + 1 < HP:
                        qts[hp + 1], kts[hp + 1], pending = make_proj_thunks(hp + 1)
                    qA = qt_t[0:64, qt * 512:(qt + 1) * 512]
                    qB = qt_t[64:128, qt * 512:(qt + 1) * 512]
                    # Software-pipelined by one chunk: emit scores(kc) and its
                    # exp, then the AV matmuls for kc-1 — so the PE always has
                    # independent score work queued while ScalarE runs exp.
                    # A few exp tiles per pass run on VectorE instead
                    # (dve_exp); their AV is deferred 4 chunks for pipeline
                    # depth. None in (hp0,qt0)/(hp0,qt1) where VectorE is
                    # busy with v/kt/qt-projection evacuations.
                    # DVE-exp offload measured net-negative (VectorE
                    # elementwise ~1.2us/pass for [128,1024] fp32, chain ~8us
                    # vs ScalarE 1.1us) -- keep disabled.
                    dve_kcs = ()
                    ets = {}
                    av_due = []  # (kc, due-iteration), emission-ordered

                    def av_pair(kc, rA=rA, rB=rB, ets=ets, h0=h0, h1=h1):
                        vt = vones[kc]
                        et = ets.pop(kc)
                        nc.tensor.matmul(
                            rA[0:65, :],
                            vt[:, h0 * 65:(h0 + 1) * 65],
                            et[:, 0:512],
                            start=(kc == 0), stop=(kc == KC - 1),
                        )
                        nc.tensor.matmul(
                            rB[0:65, :],
                            vt[:, h1 * 65:(h1 + 1) * 65],
                            et[:, 512:1024],
                            start=(kc == 0), stop=(kc == KC - 1),
                        )

                    for kc in range(KC):
                        sp = pss.tile([P, 1024], f32, tag="sc")
                        nc.tensor.matmul(
                            sp[:, 0:512],
                            kt_t[0:64, kc * P:(kc + 1) * P],
                            qA,
                            start=True, stop=True,
                            tile_position=(0, 0),
                        )
                        nc.tensor.matmul(
                            sp[:, 512:1024],
                            kt_t[64:128, kc * P:(kc + 1) * P],
                            qB,
                            start=True, stop=True,
                            tile_position=(64, 0),
                        )
                        et = etp.tile([P, 1024], bf16, tag="et")
                        if kc in dve_kcs:
                            dve_exp(sp, et)
                            av_due.append((kc, kc + 4))
                        else:
                            nc.scalar.activation(et[:], sp[:], AF.Exp,
                                                 scale=float(SCALE))
                            av_due.append((kc, kc + 1))
                        ets[kc] = et
                        if hp == 0 and qt == 0:
                            # JIT v projection: vone[kc] lands just ahead of
                            # av_pair(kc); emitting it after scores/exp keeps
                            # the first exp off the wv-DMA critical path.
                            vproj_group(kc)
                            if hp0_kt and kc % 4 == 2:
                                hp0_kt.pop(0)()
                            if kc == 24:
                                for t in hp0_qt1_proj:
                                    t()
                                hp0_qt1_proj = []
                        for ent in [e for e in av_due if e[1] <= kc]:
                            av_due.remove(ent)
                            av_pair(ent[0])
                        if tail and kc in TAIL_KCS:
                            tail.pop(0)()
                        if pending and kc % 3 == 2:
                            pending.pop(0)()
                    for ent in av_due:  # drain (ends with kc=KC-1: stop flag)
                        av_pair(ent[0])
                    for t in tail:  # safety drain (normally empty by now)
                        t()
                    tail = []
                    # Evacuate both accumulators to SBUF immediately (frees
                    # the PSUM slots); rowsums go to a [2, 512] tile for one
                    # fast approximate reciprocal over both heads.
                    rsbA = nrm.tile([65, 512], f32, tag="rsb", bufs=4, name="rsbA")
                    nc.vector.tensor_copy(rsbA[:], rA[0:65, :])
                    rsbB = nrm.tile([65, 512], f32, tag="rsb", bufs=4, name="rsbB")
                    nc.vector.tensor_copy(rsbB[:], rB[0:65, :])
                    scr = nrm.tile([1, 1024], f32, tag="scr", bufs=1, name="scr")
                    nc.vector.tensor_copy(scr[0:1, 0:512], rsbA[64:65, :])
                    nc.vector.tensor_copy(scr[0:1, 512:1024], rsbB[64:65, :])
                    srec = nrm.tile([1, 1024], f32, tag="srec", bufs=2, name="srec")
                    nc.vector.reciprocal_approx_fast(srec[:], scr[:])
                    tail = make_tail(hp, qt, rsbA, rsbB, srec)

            for t in tail:  # final iteration's tail
                t()

    nc.compile()
    return nc


def _get_program():
    global _PROGRAM
    if _PROGRAM is None:
        _PROGRAM = _build_program()
    return _PROGRAM


def kernel(x, Wq, Wk, Wv, Wo, bo, gamma_q, gamma_k, gamma_v, gamma_out):
    from concourse import bass_utils

    import ml_dtypes

    bf16 = ml_dtypes.bfloat16
    x = np.asarray(x, dtype=np.float32)
    f32 = np.float32
    WqT = np.ascontiguousarray((np.asarray(Wq, f32).T * np.asarray(gamma_q, f32)[None, :]).astype(bf16))
    WkT = np.ascontiguousarray((np.asarray(Wk, f32).T * np.asarray(gamma_k, f32)[None, :]).astype(bf16))
    WvT = np.ascontiguousarray((np.asarray(Wv, f32).T * np.asarray(gamma_v, f32)[None, :]).astype(bf16))
    WoT = np.ascontiguousarray((np.asarray(Wo, f32).T * np.asarray(gamma_out, f32)[None, :]).astype(bf16))
    bo_s = np.ascontiguousarray(np.asarray(gamma_out, f32) * np.asarray(bo, f32))

    xT = np.ascontiguousarray(x.transpose(0, 2, 1).astype(bf16))  # [B, D, N]

    in_maps = []
    for c in range(NCORES):
        b, q0 = c // 4, (c % 4) * QPC
        in_maps.append({
            "xT": xT[b],
            "xTq": np.ascontiguousarray(xT[b][:, q0:q0 + QPC]),
            "wqT": WqT, "wkT": WkT, "wvT": WvT, "woT": WoT,
            "bo": bo_s,
        })

    nc = _get_program()
    res = bass_utils.run_bass_kernel_spmd(nc, in_maps, core_ids=list(range(NCORES)))
    global LAST_RESULT
    LAST_RESULT = res

    out = np.empty((B, N, D), np.float32)
    for c in range(NCORES):
        b, q0 = c // 4, (c % 4) * QPC
        out[b, q0:q0 + QPC, :] = res.results[c]["outT"].T
    return out


# revision 23
# speedup vs baseline: 1.3213x; 1.0135x over previous
"""Trainium2 Bass kernel: fused multi-head self-attention block (CrossAttention module).

Sharding: 8 cores, each handles one (batch, query-slice) pair:
  core c -> batch b = c // 4, query rows q0 = (c % 4) * 1024 .. +1024.
Each core computes K/V projections for its full batch (replicated across the 4
cores sharing a batch), Q projection for its query slice, all 8 heads of
attention for its queries, and the output projection for its rows.
Host folds the per-channel gammas into the (pre-transposed) weights, transposes
x once, and concatenates the per-core outputs.

On-chip dataflow (per core, all fp32 accumulation, bf16 operands):
  - xT / xTq staged resident in SBUF once (no inner-loop DMA)
  - kT[ko, n]  = WkT.T @ xT   (key channels on partitions)  -- JIT per head-pair
  - qT[qo, n]  = WqT.T @ xTq                                -- JIT per head-pair
  - v[k, vo]   = xT.T @ WvT, stored interleaved with a ones column per head
                 ("vone" [128, 8*65]) so the attention rowsum comes free
  - scoresT tile [key 128, q 512] = kT_h.T @ qT_h, two heads packed as PE
    row-tiles (K=64 each) into one 2-bank PSUM tile (concurrent execution)
  - E = exp(SCALE * scoresT) via ScalarE, PSUM -> SBUF ([128, 1024] per instr)
  - rT[dv(+rowsum), q] += vone_h.T @ E, accumulated over 32 key chunks in PSUM
  - normalize: rowsums copied to a [2, 512] tile, reciprocal_approx_fast (DVE),
    partition-broadcast via a K=2 ones-matmul on the PE, then one DVE multiply
    per head -- no slow RECIPROCAL, no GpSimd on the critical path
  - outT[do, q] = WoT.T @ rTn + bo, accumulated across head pairs in SBUF
  - the whole normalize+output-projection tail is emitted as deferred thunks
    interleaved into the NEXT (hp, qt) iteration's key loop, so the PE queue
    never stalls at an iteration boundary (keeps HAM at full clock)
"""

import os
import sys

import numpy as np

for _p in ("/opt/trn_rl_repo", "/root/.axon_site/_ro/trn_rl_repo"):
    if os.path.isdir(_p) and _p not in sys.path:
        sys.path.append(_p)

B, N, D = 2, 4096, 512
H, DH = 8, 64
SCALE = DH ** -0.5
NCORES = 8
QPC = (B * N) // NCORES  # 1024 query rows per core
P = 128
CD = D // P              # 4 contraction chunks of 128
KC = N // P              # 32 key chunks of 128
NT = N // 512            # 8 key-column tiles of 512
QT = QPC // 512          # 2 query tiles of 512
HP = H // 2              # 4 head pairs

_PROGRAM = None
LAST_RESULT = None

# DVE-exp offload (Schraudolph magic-add + shift-bitcast 2^i, cubic 2^f):
# exp(SCALE*s) = 2^(A2*s); u = t+MAGIC2 rounds t to int i in u's mantissa,
# (bits(u)<<23) = bits(2^i) (127 bias folded into MAGIC2), f = t-i in
# [-0.5,0.5], 2^f ~ C0+C1*f+C2*f^2+C3*f^3 (max rel err 1.4e-4, under the
# bf16 output quantization). Runs on the otherwise-idle VectorE to relieve
# the saturated ScalarE exp stream.
import math
A2 = SCALE * math.log2(math.e)
MAGIC2 = float(1.5 * 2 ** 23 + 127)
EC0, EC1, EC2, EC3 = 0.99995134, 0.69325305, 0.24225698, 0.05502927


def _build_program():
    import concourse.tile as tile
    from concourse import bacc, mybir

    f32 = mybir.dt.float32
    bf16 = mybir.dt.bfloat16
    i32 = mybir.dt.int32
    AF = mybir.ActivationFunctionType
    OP = mybir.AluOpType

    nc = bacc.Bacc("TRN2", target_bir_lowering=False, debug=False)

    xT_a = nc.dram_tensor("xT", [D, N], bf16, kind="ExternalInput").ap()
    xTq_a = nc.dram_tensor("xTq", [D, QPC], bf16, kind="ExternalInput").ap()
    wq_a = nc.dram_tensor("wqT", [D, D], bf16, kind="ExternalInput").ap()
    wk_a = nc.dram_tensor("wkT", [D, D], bf16, kind="ExternalInput").ap()
    wv_a = nc.dram_tensor("wvT", [D, D], bf16, kind="ExternalInput").ap()
    wo_a = nc.dram_tensor("woT", [D, D], bf16, kind="ExternalInput").ap()
    bo_a = nc.dram_tensor("bo", [D], f32, kind="ExternalInput").ap()
    outT_a = nc.dram_tensor("outT", [D, QPC], f32, kind="ExternalOutput").ap()

    with tile.TileContext(nc) as tc:
        with (
            tc.tile_pool(name="w", bufs=1) as wpool,
            tc.tile_pool(name="xr", bufs=1) as xr,
            tc.tile_pool(name="kT", bufs=2) as kTp,
            tc.tile_pool(name="qT", bufs=2) as qTp,
            tc.tile_pool(name="vone", bufs=1) as vpool,
            tc.tile_pool(name="et", bufs=6) as etp,
            tc.tile_pool(name="dx", bufs=1) as dxp,
            tc.tile_pool(name="rTn", bufs=1) as rTnp,
            tc.tile_pool(name="ot", bufs=2) as otp,
            tc.tile_pool(name="nrm", bufs=2) as nrm,
            tc.tile_pool(name="acc", bufs=2, space="PSUM") as psa,
            tc.tile_pool(name="sc", bufs=2, space="PSUM") as pss,
        ):
            # ---- resident inputs: x (transposed), q-slice of x, weights.
            # One coalesced DMA per 512-column chunk / weight, emitted in
            # dependency order (wk+wq+xTq0+wv ahead of the bulk of xT) and
            # spread over both hardware DGE queues (SP + ACT) so the first
            # scores matmul can issue a few microseconds in.
            xT_sb = xr.tile([P, CD * N], bf16, tag="xT")
            xTq_sb = xr.tile([P, CD * QPC], bf16, tag="xTq")
            xT_src = xT_a.rearrange("(c p) n -> p c n", p=P)
            xT_dst = xT_sb[:].rearrange("p (c n) -> p c n", c=CD)
            xTq_src = xTq_a.rearrange("(c p) n -> p c n", p=P)
            xTq_dst = xTq_sb[:].rearrange("p (c n) -> p c n", c=CD)

            def wtile(tag):
                return wpool.tile([P, CD * 512], bf16, tag=tag, name=tag)

            wk, wq, wv, wo = wtile("wk"), wtile("wq"), wtile("wvo"), wtile("wo")
            bo_t = wpool.tile([P, CD], f32, tag="bo")

            def w_dma(eng, w, dram_ap, lo, hi):
                eng.dma_start(
                    w[:].rearrange("p (c n) -> p c n", c=CD)[:, :, lo:hi],
                    dram_ap.rearrange("(c p) n -> p c n", p=P)[:, :, lo:hi],
                )

            # Per-queue DMA is ~90-110 GB/s, so three DMA-issuing queues
            # (SP + ACT hardware DGE, GpSimd software DGE) are loaded in
            # strict need-order: head-pair-0 slices of wq/wk and the first x
            # chunks (split in cd halves so the first proj matmuls can chase
            # the DMAs) ahead of everything else.
            w_dma(nc.sync, wq, wq_a, 0, D)
            nc.scalar.dma_start(xT_dst[:, :, 0:512], xT_src[:, :, 0:512])
            nc.sync.dma_start(xTq_dst[:, :, 0:512], xTq_src[:, :, 0:512])
            w_dma(nc.sync, wk, wk_a, 0, D)
            nc.scalar.dma_start(xT_dst[:, :, 512:1024], xT_src[:, :, 512:1024])
            w_dma(nc.sync, wv, wv_a, 0, D)
            nc.sync.dma_start(xTq_dst[:, :, 512:1024], xTq_src[:, :, 512:1024])
            for nt in range(2, NT):
                nc.scalar.dma_start(
                    xT_dst[:, :, nt * 512:(nt + 1) * 512],
                    xT_src[:, :, nt * 512:(nt + 1) * 512],
                )
            w_dma(nc.sync, wo, wo_a, 0, D)
            nc.sync.dma_start(bo_t[:], bo_a.rearrange("(c p) -> p c", p=P))

            def dve_exp(sp, et):
                """8-pass VectorE exp(SCALE*x): see module docstring constants."""
                u = dxp.tile([P, 1024], f32, tag="a", name="dveu")
                nc.vector.tensor_scalar(u[:], sp[:], A2, MAGIC2, OP.mult, OP.add)
                i_ = dxp.tile([P, 1024], f32, tag="b", name="dvei")
                nc.vector.tensor_scalar(i_[:], u[:], MAGIC2, None, OP.subtract)
                fT = dxp.tile([P, 1024], f32, tag="c", name="dvef")
                nc.vector.scalar_tensor_tensor(
                    fT[:], sp[:], A2, i_[:], OP.mult, OP.subtract)
                pb = dxp.tile([P, 1024], i32, tag="d", name="dvepb")
                nc.vector.tensor_scalar(
                    pb[:], u[:].bitcast(i32), 23, None, OP.logical_shift_left)
                h1 = dxp.tile([P, 1024], f32, tag="a", name="dveh1")
                nc.vector.tensor_scalar(h1[:], fT[:], EC3, EC2, OP.mult, OP.add)
                h2 = dxp.tile([P, 1024], f32, tag="b", name="dveh2")
                nc.vector.tensor_tensor(h2[:], h1[:], fT[:], op=OP.mult)
                h4 = dxp.tile([P, 1024], f32, tag="a", name="dveh4")
                nc.vector.scalar_tensor_tensor(
                    h4[:], h2[:], EC1, fT[:], OP.add, OP.mult)
                nc.vector.scalar_tensor_tensor(
                    et[:], h4[:], EC0, pb[:].bitcast(f32), OP.add, OP.mult)

            vones = [None] * KC
            oaccs = {}
            rTns = [
                rTnp.tile([P, QPC], bf16, tag=f"rTn{c}", name=f"rTn{c}")
                for c in range(CD)
            ]

            def proj_group(w_t, hp, src_sb, nt, dst):
                """One 512-wide output block of a W.T @ x projection:
                4 accumulating matmuls off the resident x, 1 evacuation."""
                src_stride = N if src_sb is xT_sb else QPC
                ps = psa.tile([P, 512], f32, tag="acc")
                for cd in range(CD):
                    nc.tensor.matmul(
                        ps[:],
                        w_t[:, cd * 512 + hp * P: cd * 512 + (hp + 1) * P],
                        src_sb[:, cd * src_stride + nt * 512:
                               cd * src_stride + (nt + 1) * 512],
                        start=(cd == 0),
                        stop=(cd == CD - 1),
                    )
                nc.vector.tensor_copy(dst, ps[:])

            def vproj_group(kc):
                """v projection for one 128-key chunk, written into the
                per-head [64 v | 1 ones] interleaved layout."""
                ps = psa.tile([P, 512], f32, tag="acc")
                for cd in range(CD):
                    nc.tensor.matmul(
                        ps[:],
                        xT_sb[:, cd * N + kc * P: cd * N + (kc + 1) * P],
                        wv[:, cd * 512:(cd + 1) * 512],
                        start=(cd == 0),
                        stop=(cd == CD - 1),
                    )
                vt = vpool.tile([P, H * 65], bf16, tag=f"vone{kc}")
                v3 = vt[:].rearrange("p (h c) -> p h c", c=65)
                nc.vector.tensor_copy(
                    v3[:, :, 0:64], ps[:].rearrange("p (h c) -> p h c", c=64)
                )
                nc.vector.memset(v3[:, :, 64:65], 1.0)
                vones[kc] = vt

            def make_proj_thunks(hp):
                qt_t = qTp.tile([P, QPC], bf16, tag="qT")
                kt_t = kTp.tile([P, N], bf16, tag="kT")
                thunks = []
                for nt in range(QT):
                    thunks.append(
                        lambda nt=nt, qt_t=qt_t, hp=hp: proj_group(
                            wq, hp, xTq_sb, nt, qt_t[:, nt * 512:(nt + 1) * 512]
                        )
                    )
                for nt in range(NT):
                    thunks.append(
                        lambda nt=nt, kt_t=kt_t, hp=hp: proj_group(
                            wk, hp, xT_sb, nt, kt_t[:, nt * 512:(nt + 1) * 512]
                        )
                    )
                return qt_t, kt_t, thunks

            def make_tail(hp, qt, rsbA, rsbB, srec):
                """Deferred normalize + output-projection thunks for (hp, qt).
                Emitted interleaved into the next iteration's key loop so the
                PE queue keeps streaming scores/AV while the DVE chain runs."""
                thunks = []

                def bcast(hp=hp, qt=qt, srec=srec):
                    # broadcast 1/rowsum across partitions on the (otherwise
                    # idle) GpSimd engine; latency hidden by the deferred-tail
                    # interleaving.
                    bcA = nrm.tile([64, 512], f32, tag="bcA", bufs=1, name="bcA")
                    nc.gpsimd.partition_broadcast(bcA[:], srec[0:1, 0:512])
                    bcB = nrm.tile([64, 512], f32, tag="bcB", bufs=1, name="bcB")
                    nc.gpsimd.partition_broadcast(bcB[:], srec[0:1, 512:1024])
                    for rsb, bc, poff in ((rsbA, bcA, 0), (rsbB, bcB, 64)):
                        nc.vector.tensor_tensor(
                            rTns[hp][poff:poff + 64, qt * 512:(qt + 1) * 512],
                            rsb[0:64, :],
                            bc[:],
                            op=OP.mult,
                        )

                thunks.append(bcast)

                def oproj(doc, hp=hp, qt=qt):
                    ps = psa.tile([P, 512], f32, tag="acc", name="ops")
                    nc.tensor.matmul(
                        ps[:],
                        wo[:, hp * 512 + doc * P: hp * 512 + (doc + 1) * P],
                        rTns[hp][:, qt * 512:(qt + 1) * 512],
                        start=True, stop=True,
                    )
                    if hp == 0:
                        oa = otp.tile([P, 512], f32, tag=f"oacc{qt}{doc}",
                                      bufs=1, name=f"oacc{qt}{doc}")
                        oaccs[(qt, doc)] = oa
                        nc.vector.tensor_copy(oa[:], ps[:])
                    else:
                        oa = oaccs[(qt, doc)]
                        nc.vector.tensor_tensor(oa[:], oa[:], ps[:], op=OP.add)
                    if hp == HP - 1:
                        ot = otp.tile([P, 512], f32, tag="ot")
                        nc.vector.tensor_tensor(
                            ot[:],
                            oa[:],
                            bo_t[:, doc:doc + 1].to_broadcast((P, 512)),
                            op=OP.add,
                        )
                        nc.sync.dma_start(
                            outT_a[doc * P:(doc + 1) * P,
                                   qt * 512:(qt + 1) * 512],
                            ot[:],
                        )

                for doc in range(CD):
                    thunks.append(lambda doc=doc: oproj(doc))
                return thunks

            qts, kts = {}, {}
            qts[0], kts[0], th0 = make_proj_thunks(0)
            # Emit only the blocks needed to start attention: qT block 0 and
            # kT block 0; the rest of hp0's projections interleave into the
            # first kc loop (kT block g must land before kc reaches 4g).
            th0[0]()
            th0[QT]()
            hp0_qt1_proj = th0[1:QT]
            hp0_kt = th0[QT + 1:]
            pending = []
            tail = []
            TAIL_KCS = (3, 5, 7, 9, 11)

            for hp in range(HP):
                qt_t, kt_t = qts[hp], kts[hp]
                for t in pending:  # leftover projections for this head pair
                    t()
                pending = []
                h0, h1 = 2 * hp, 2 * hp + 1
                for qt in range(QT):
                    rA = psa.tile([P, 512], f32, tag="rA", bufs=1, name="rA")
                    rB = psa.tile([P, 512], f32, tag="rB", bufs=1, name="rB")
                    if qt == 1 and hp + 1 < HP:
                        qts[hp + 1], kts[hp + 1], pending = make_proj_thunks(hp + 1)
                    qA = qt_t[0:64, qt * 512:(qt + 1) * 512]
                    qB = qt_t[64:128, qt * 512:(qt + 1) * 512]
                    # Software-pipelined by one chunk: emit scores(kc) and its
                    # exp, then the AV matmuls for kc-1 — so the PE always has
                    # independent score work queued while ScalarE runs exp.
                    # A few exp tiles per pass run on VectorE instead
                    # (dve_exp); their AV is deferred 4 chunks for pipeline
                    # depth. None in (hp0,qt0)/(hp0,qt1) where VectorE is
                    # busy with v/kt/qt-projection evacuations.
                    # DVE-exp offload measured net-negative (VectorE
                    # elementwise ~1.2us/pass for [128,1024] fp32, chain ~8us
                    # vs ScalarE 1.1us) -- keep disabled.
                    dve_kcs = ()
                    ets = {}
                    av_due = []  # (kc, due-iteration), emission-ordered

                    def av_pair(kc, rA=rA, rB=rB, ets=ets, h0=h0, h1=h1):
                        vt = vones[kc]
                        et = ets.pop(kc)
                        nc.tensor.matmul(
                            rA[0:65, :],
                            vt[:, h0 * 65:(h0 + 1) * 65],
                            et[:, 0:512],
                            start=(kc == 0), stop=(kc == KC - 1),
                        )
                        nc.tensor.matmul(
                            rB[0:65, :],
                            vt[:, h1 * 65:(h1 + 1) * 65],
                            et[:, 512:1024],
                            start=(kc == 0), stop=(kc == KC - 1),
                        )

                    for kc in range(KC):
                        sp = pss.tile([P, 1024], f32, tag="sc")
                        nc.tensor.matmul(
                            sp[:, 0:512],
                            kt_t[0:64, kc * P:(kc + 1) * P],
                            qA,
                            start=True, stop=True,
                            tile_position=(0, 0),
                        )
                        nc.tensor.matmul(
                            sp[:, 512:1024],
                            kt_t[64:128, kc * P:(kc + 1) * P],
                            qB,
                            start=True, stop=True,
                            tile_position=(64, 0),
                        )
                        et = etp.tile([P, 1024], bf16, tag="et")
                        if kc in dve_kcs:
                            dve_exp(sp, et)
                            av_due.append((kc, kc + 4))
                        else:
                            nc.scalar.activation(et[:], sp[:], AF.Exp,
                                                 scale=float(SCALE))
                            av_due.append((kc, kc + 1))
                        ets[kc] = et
                        if hp == 0 and qt == 0:
                            # JIT v projection: vone[kc] lands just ahead of
                            # av_pair(kc); emitting it after scores/exp keeps
                            # the first exp off the wv-DMA critical path.
                            vproj_group(kc)
                            if hp0_kt and kc % 4 == 2:
                                hp0_kt.pop(0)()
                            if kc == 24:
                                for t in hp0_qt1_proj:
                                    t()
                                hp0_qt1_proj = []
                        for ent in [e for e in av_due if e[1] <= kc]:
                            av_due.remove(ent)
                            av_pair(ent[0])
                        if tail and kc in TAIL_KCS:
                            tail.pop(0)()
                        if pending and kc % 3 == 2:
                            pending.pop(0)()
                    for ent in av_due:  # drain (ends with kc=KC-1: stop flag)
                        av_pair(ent[0])
                    for t in tail:  # safety drain (normally empty by now)
                        t()
                    tail = []
                    # Evacuate both accumulators to SBUF immediately (frees
                    # the PSUM slots); rowsums go to a [2, 512] tile for one
                    # fast approximate reciprocal over both heads.
                    rsbA = nrm.tile([65, 512], f32, tag="rsb", bufs=4, name="rsbA")
                    nc.vector.tensor_copy(rsbA[:], rA[0:65, :])
                    rsbB = nrm.tile([65, 512], f32, tag="rsb", bufs=4, name="rsbB")
                    nc.vector.tensor_copy(rsbB[:], rB[0:65, :])
                    scr = nrm.tile([1, 1024], f32, tag="scr", bufs=1, name="scr")
                    nc.vector.tensor_copy(scr[0:1, 0:512], rsbA[64:65, :])
                    nc.vector.tensor_copy(scr[0:1, 512:1024], rsbB[64:65, :])
                    srec = nrm.tile([1, 1024], f32, tag="srec", bufs=2, name="srec")
                    nc.vector.reciprocal_approx_fast(srec[:], scr[:])
                    tail = make_tail(hp, qt, rsbA, rsbB, srec)

            for t in tail:  # final iteration's tail
                t()

    nc.compile()
    return nc


def _get_program():
    global _PROGRAM
    if _PROGRAM is None:
        _PROGRAM = _build_program()
    return _PROGRAM


def kernel(x, Wq, Wk, Wv, Wo, bo, gamma_q, gamma_k, gamma_v, gamma_out):
    from concourse import bass_utils

    import ml_dtypes

    bf16 = ml_dtypes.bfloat16
    x = np.asarray(x, dtype=np.float32)
    f32 = np.float32
    WqT = np.ascontiguousarray((np.asarray(Wq, f32).T * np.asarray(gamma_q, f32)[None, :]).astype(bf16))
    WkT = np.ascontiguousarray((np.asarray(Wk, f32).T * np.asarray(gamma_k, f32)[None, :]).astype(bf16))
    WvT = np.ascontiguousarray((np.asarray(Wv, f32).T * np.asarray(gamma_v, f32)[None, :]).astype(bf16))
    WoT = np.ascontiguousarray((np.asarray(Wo, f32).T * np.asarray(gamma_out, f32)[None, :]).astype(bf16))
    bo_s = np.ascontiguousarray(np.asarray(gamma_out, f32) * np.asarray(bo, f32))

    xT = np.ascontiguousarray(x.transpose(0, 2, 1).astype(bf16))  # [B, D, N]

    in_maps = []
    for c in range(NCORES):
        b, q0 = c // 4, (c % 4) * QPC
        in_maps.append({
            "xT": xT[b],
            "xTq": np.ascontiguousarray(xT[b][:, q0:q0 + QPC]),
            "wqT": WqT, "wkT": WkT, "wvT": WvT, "woT": WoT,
            "bo": bo_s,
        })

    nc = _get_program()
    res = bass_utils.run_bass_kernel_spmd(nc, in_maps, core_ids=list(range(NCORES)))
    global LAST_RESULT
    LAST_RESULT = res

    out = np.empty((B, N, D), np.float32)
    for c in range(NCORES):
        b, q0 = c // 4, (c % 4) * QPC
        out[b, q0:q0 + QPC, :] = res.results[c]["outT"].T
    return out
